# revision 9
# baseline (speedup 1.0000x reference)
"""Sliding-window (radius-8, K=17) single-head attention along W — v3.

Math (see host_consts): S[w',w] = g(w')·x(w) + alpha(w') (+terms that cancel
in softmax), g = M x + u with M = Wq^T Wk/sc; alpha rides the ACT exp bias;
the global constant c0 and fp16-range shift s0 are baked into the zero-pad
denominator term. Banded 136-col score/den/value matmuls in fp16.

Engine assignment per row of [C=128, W=256]:
  Sync   : input f/p chunk DMAs (HWDGE), x^T via X-bar DMA-transpose, out DMA
  GpSimd : x = f + p (f32+f32 -> fp16), att band-mask multiply
  PE     : g projection (N=512/pair), alpha (N=1), banded scores, den, value
  ACT    : exp with per-chunk alpha bias, alpha eviction (-s0)
  DVE    : g eviction (+u, ->fp16), reciprocal of den, final normalize mul
"""

import numpy as np
from contextlib import ExitStack

import concourse.bacc as bacc
import concourse.mybir as mybir
import concourse.tile as tile
from concourse.bass_utils import run_bass_kernel_spmd

B, C, H, W = 2, 128, 64, 256
R = 8
NCORES = 8
ROWS = B * H // NCORES        # 16 (b, h) rows per core
CORES_PER_B = NCORES // B     # 4
F32 = mybir.dt.float32
F16 = mybir.dt.float16
EXP = mybir.ActivationFunctionType.Exp
MULT = mybir.AluOpType.mult
S0 = 7.0                      # constant score shift (softmax-invariant)
NB = 136                      # banded block width (128 + R)


def build_nc():
    nc = bacc.Bacc(trn_type="TRN2")
    f_ext = nc.dram_tensor("feature", [C, ROWS, W], F32, kind="ExternalInput")
    p_ext = nc.dram_tensor("position", [C, ROWS, W], F32, kind="ExternalInput")
    mt_ext = nc.dram_tensor("mt", [C, C], F16, kind="ExternalInput")
    ones_ext = nc.dram_tensor("ones", [C, C], F16, kind="ExternalInput")
    v_ext = nc.dram_tensor("vt", [C, 1], F16, kind="ExternalInput")
    u_ext = nc.dram_tensor("ut", [C, 1], F32, kind="ExternalInput")
    band_ext = nc.dram_tensor("band01", [C, 2 * NB], F16, kind="ExternalInput")
    oob_ext = nc.dram_tensor("oob16", [C, W], F16, kind="ExternalInput")
    out_ext = nc.dram_tensor("out", [C, ROWS, W], F16, kind="ExternalOutput")

    with tile.TileContext(nc) as tc, ExitStack() as ctx:
        const = ctx.enter_context(tc.tile_pool(name="const", bufs=1))
        xgp = ctx.enter_context(tc.tile_pool(name="xg", bufs=1))
        inp = ctx.enter_context(tc.tile_pool(name="inp", bufs=2))

        # input chunks first so compute can start as early as possible
        CH = 4 * W
        x_sb = xgp.tile([C, ROWS * W], F16, tag="x")
        fts, pts = [], []
        for c4 in range(ROWS // 4):
            ft = inp.tile([C, CH], F32, tag="ft")
            nc.sync.dma_start(ft[:], f_ext[:, 4 * c4 : 4 * c4 + 4, :])
            pt = inp.tile([C, CH], F32, tag="pt")
            nc.sync.dma_start(pt[:], p_ext[:, 4 * c4 : 4 * c4 + 4, :])
            fts.append(ft)
            pts.append(pt)

        def cload(shape, dt, ext, tag):
            t = const.tile(shape, dt, tag=tag, name=tag)
            nc.sync.dma_start(t[:], ext[:])
            return t

        mt_t = cload([C, C], F16, mt_ext, "mt")
        ones_t = cload([C, C], F16, ones_ext, "ones")
        v_t = cload([C, 1], F16, v_ext, "v")
        u_t = cload([C, 1], F32, u_ext, "u")
        band_t = cload([C, 2 * NB], F16, band_ext, "band")
        oob_t = cload([C, W], F16, oob_ext, "oob")

        # x = f + p as fp16 (gpsimd), one op per 4-row chunk
        for c4 in range(ROWS // 4):
            sl = slice(c4 * CH, (c4 + 1) * CH)
            nc.gpsimd.tensor_add(x_sb[:, sl], fts[c4][:], pts[c4][:])

        # touch Exp once so the ACT table loads during the input-DMA ramp
        warm = const.tile([C, 1], F32, tag="warm")
        nc.scalar.activation(warm[:], u_t[:], EXP)
        negs0 = const.tile([C, 1], F32, tag="negs0")
        nc.vector.memset(negs0[:], -S0)

        g_sb = xgp.tile([C, ROWS * W], F16, tag="g")

        attp = ctx.enter_context(tc.tile_pool(name="att", bufs=3))
        sbp = ctx.enter_context(tc.tile_pool(name="sb", bufs=2))
        ps_s = ctx.enter_context(tc.tile_pool(name="ps_s", bufs=2, space="PSUM"))
        ps_g = ctx.enter_context(tc.tile_pool(name="ps_g", bufs=2, space="PSUM"))
        ps_dn = ctx.enter_context(tc.tile_pool(name="ps_dn", bufs=2, space="PSUM"))
        ps_o = ctx.enter_context(tc.tile_pool(name="ps_o", bufs=2, space="PSUM"))

        for half in range(2):          # 8-row halves; g computed per half
            r0 = 8 * half
            # g = M x (+u at eviction): one N=512 matmul per row-pair,
            # M^T stationary loaded once per half.
            for pr in range(4):
                r = r0 + 2 * pr
                xsl = slice(r * W, (r + 2) * W)
                g_ps = ps_g.tile([C, 2 * W], F32, tag="g")
                nc.tensor.matmul(g_ps[:], mt_t[:], x_sb[:, xsl], start=True, stop=True)
                nc.vector.tensor_scalar_add(g_sb[:, xsl], g_ps[:], u_t[:])

            for pr in range(4):
                r = r0 + 2 * pr
                den_ps = ps_dn.tile([C, 2 * W], F32, tag="dn")
                out_ps = ps_o.tile([C, 2 * W], F32, tag="out")
                # x^T for both rows via X-bar DMA transpose:
                # xt_sb[p, i, c] = x[c, i*128 + p] (i: 2 chunks x 2 rows)
                xt_sb = sbp.tile([C, 4, 128], F16, tag="xt")
                nc.sync.dma_start_transpose(
                    xt_sb[:], x_sb[:, r * W : (r + 2) * W]
                )
                atts = []
                for rr in range(2):
                    x_r = x_sb[:, (r + rr) * W : (r + rr + 1) * W]
                    g_r = g_sb[:, (r + rr) * W : (r + rr + 1) * W]
                    o0 = rr * W

                    s_ps = ps_s.tile([C, 2 * NB + 2], F32, tag="s")
                    # alpha = v^T x per key chunk (x chunk is the stationary)
                    nc.tensor.matmul(
                        s_ps[:, 2 * NB : 2 * NB + 1], x_r[:, 0:128], v_t[:],
                        start=True, stop=True,
                    )
                    nc.tensor.matmul(
                        s_ps[:, 2 * NB + 1 : 2 * NB + 2], x_r[:, 128:256], v_t[:],
                        start=True, stop=True,
                    )
                    # banded scores: S^T block per key chunk
                    nc.tensor.matmul(
                        s_ps[:, 0:NB], g_r[:, 0:128], x_r[:, 0:NB],
                        start=True, stop=True,
                    )
                    nc.tensor.matmul(
                        s_ps[:, NB : 2 * NB], g_r[:, 128:256], x_r[:, W - NB : W],
                        start=True, stop=True,
                    )

                    # alpha - s0 to SBUF for the exp bias (ACT)
                    al_sb = sbp.tile([C, 2], F32, tag="al")
                    nc.scalar.add(al_sb[:], s_ps[:, 2 * NB : 2 * NB + 2], negs0[:])

                    att = attp.tile([C, 2 * NB], F16, tag="att")
                    nc.scalar.activation(
                        att[:, 0:NB], s_ps[:, 0:NB], EXP, bias=al_sb[:, 0:1]
                    )
                    nc.scalar.activation(
                        att[:, NB : 2 * NB], s_ps[:, NB : 2 * NB], EXP,
                        bias=al_sb[:, 1:2],
                    )
                    # zero out-of-band entries
                    attm = attp.tile([C, 2 * NB], F16, tag="attm")
                    nc.gpsimd.tensor_mul(attm[:], att[:], band_t[:])
                    atts.append(attm)

                    # denominator (broadcast to all partitions via ones^T);
                    # zero-pad contribution pre-baked in oob16.
                    nc.tensor.matmul(
                        den_ps[:, o0 : o0 + W], ones_t[:], oob_t[:],
                        start=True, stop=False,
                    )
                    nc.tensor.matmul(
                        den_ps[:, o0 : o0 + NB], ones_t[:], attm[:, 0:NB],
                        start=False, stop=False,
                    )
                    nc.tensor.matmul(
                        den_ps[:, o0 + W - NB : o0 + W], ones_t[:], attm[:, NB : 2 * NB],
                        start=False, stop=True,
                    )

                for rr in range(2):
                    attm = atts[rr]
                    o0 = rr * W
                    xt0 = xt_sb[:, 2 * rr, :]
                    xt1 = xt_sb[:, 2 * rr + 1, :]
                    # out_u = x @ attU via x^T chunks as stationaries
                    nc.tensor.matmul(
                        out_ps[:, o0 : o0 + NB], xt0, attm[:, 0:NB],
                        start=True, stop=True,
                    )
                    nc.tensor.matmul(
                        out_ps[:, o0 + 120 : o0 + NB], xt1, attm[:, NB : NB + 16],
                        start=False, stop=True, skip_group_check=True,
                    )
                    nc.tensor.matmul(
                        out_ps[:, o0 + NB : o0 + W], xt1, attm[:, NB + 16 : 2 * NB],
                        start=True, stop=True,
                    )

                rden = sbp.tile([C, 2 * W], F32, tag="rd")
                nc.vector.reciprocal_approx_fast(out=rden[:], in_=den_ps[:])
                ostage = sbp.tile([C, 2 * W], F16, tag="ost")
                nc.vector.tensor_tensor(ostage[:], out_ps[:], rden[:], MULT)
                nc.sync.dma_start(out_ext[:, r : r + 2, :], ostage[:])

    nc.compile()
    return nc


def host_consts(Wq, bq, Wk, bk):
    sc = np.float32(np.sqrt(np.float32(C)))
    Wq = Wq.astype(np.float64)
    Wk = Wk.astype(np.float64)
    bq = bq.astype(np.float64)
    bk = bk.astype(np.float64)
    M = (Wq.T @ Wk) / sc
    v = (Wk.T @ bq) / sc
    u = (Wq.T @ bk) / sc
    c0 = float(bq @ bk) / sc

    mt = np.ascontiguousarray(M.T).astype(np.float16)      # lhsT for g = M x
    vt = v.reshape(C, 1).astype(np.float16)
    ut = u.reshape(C, 1).astype(np.float32)
    ones = np.ones((C, C), dtype=np.float16)

    # band01[p, col]: chunk0 cols 0..NB-1 (query w=col, key p),
    # chunk1 cols NB..2NB-1 (query w=120+(col-NB), key 128+p)
    band = np.zeros((C, 2 * NB), dtype=np.float16)
    for pp in range(C):
        for col in range(NB):
            if abs(col - pp) <= R:
                band[pp, col] = 1.0
            if abs((W - NB + col) - (128 + pp)) <= R:
                band[pp, NB + col] = 1.0

    wgrid = np.arange(W)
    oob_row = np.maximum(0, R - wgrid) + np.maximum(0, wgrid - (W - 1 - R))
    oob16 = np.tile(
        (oob_row * np.exp(-c0 - S0) / C).astype(np.float16), (C, 1)
    )
    return mt, vt, ut, ones, band, oob16


def core_inputs(feature, position, Wq, bq, Wk, bk):
    mt, vt, ut, ones, band, oob16 = host_consts(Wq, bq, Wk, bk)
    in_maps = []
    for i in range(NCORES):
        b = i // CORES_PER_B
        h0 = (i % CORES_PER_B) * ROWS
        in_maps.append(
            {
                "feature": np.ascontiguousarray(
                    feature[b, :, h0 : h0 + ROWS, :], dtype=np.float32
                ),
                "position": np.ascontiguousarray(
                    position[b, :, h0 : h0 + ROWS, :], dtype=np.float32
                ),
                "mt": mt,
                "ones": ones,
                "vt": vt,
                "ut": ut,
                "band01": band,
                "oob16": oob16,
            }
        )
    return in_maps


def kernel(feature, position, Wq, bq, Wk, bk):
    feature = np.asarray(feature, dtype=np.float32)
    position = np.asarray(position, dtype=np.float32)
    Wq = np.asarray(Wq, dtype=np.float32)
    bq = np.asarray(bq, dtype=np.float32)
    Wk = np.asarray(Wk, dtype=np.float32)
    bk = np.asarray(bk, dtype=np.float32)
    in_maps = core_inputs(feature, position, Wq, bq, Wk, bk)
    nc = build_nc()
    res = run_bass_kernel_spmd(nc, in_maps, list(range(NCORES)))
    out = np.empty((B, C, H, W), dtype=np.float32)
    for i in range(NCORES):
        b = i // CORES_PER_B
        h0 = (i % CORES_PER_B) * ROWS
        out[b, :, h0 : h0 + ROWS, :] = res.results[i]["out"].astype(np.float32)
    return out


# revision 10
# speedup vs baseline: 1.1828x; 1.1828x over previous
"""Sliding-window (radius-8, K=17) single-head attention along W — v4.

Math (see host_consts): S[w',w] = g(w')·x(w) + alpha(w') (+terms that cancel
in softmax), g = M x + u with M = Wq^T Wk/sc; alpha rides the ACT exp bias;
the global constant c0 and fp16-range shift s0 are baked into the zero-pad
denominator term. Banded 136-col score/den/value matmuls in fp16, with an
additive -30000 band mask accumulated into the score PSUM by a PE matmul
(exp of masked entries underflows to 0, so den/value read exp output as-is).

Engine assignment per row of [C=128, W=256]:
  Sync   : input f/p chunk DMAs (HWDGE), x^T X-bar DMA-transposes, out DMAs
  Scalar : const DMAs (parallel queue), exp with alpha bias, alpha evict
  GpSimd : x = f + p (f32+f32 -> fp16)
  PE     : g (N=512/pair), alpha (N=1), mask+scores, den, value
  DVE    : g eviction (+u -> fp16), reciprocal of den, final normalize mul
"""

import numpy as np
from contextlib import ExitStack

import concourse.bacc as bacc
import concourse.mybir as mybir
import concourse.tile as tile
from concourse.bass_utils import run_bass_kernel_spmd

B, C, H, W = 2, 128, 64, 256
R = 8
NCORES = 8
ROWS = B * H // NCORES        # 16 (b, h) rows per core
CORES_PER_B = NCORES // B     # 4
F32 = mybir.dt.float32
F16 = mybir.dt.float16
EXP = mybir.ActivationFunctionType.Exp
MULT = mybir.AluOpType.mult
S0 = 7.0                      # constant score shift (softmax-invariant)
NB = 136                      # banded block width (128 + R)
CHUNK_ROWS = [2, 2, 4, 4, 4]  # input/x chunking (small first chunks)


def build_nc():
    nc = bacc.Bacc(trn_type="TRN2")
    f_ext = nc.dram_tensor("feature", [C, ROWS, W], F32, kind="ExternalInput")
    p_ext = nc.dram_tensor("position", [C, ROWS, W], F32, kind="ExternalInput")
    mt_ext = nc.dram_tensor("mt", [C, C], F16, kind="ExternalInput")
    ones_ext = nc.dram_tensor("ones", [C, C], F16, kind="ExternalInput")
    id_ext = nc.dram_tensor("ident", [C, C], F16, kind="ExternalInput")
    v_ext = nc.dram_tensor("vt", [C, 1], F16, kind="ExternalInput")
    u_ext = nc.dram_tensor("ut", [C, 1], F32, kind="ExternalInput")
    mask_ext = nc.dram_tensor("maskT", [C, 2 * NB], F16, kind="ExternalInput")
    oob_ext = nc.dram_tensor("oob16", [C, W], F16, kind="ExternalInput")
    out_ext = nc.dram_tensor("out", [C, ROWS, W], F16, kind="ExternalOutput")

    with tile.TileContext(nc) as tc, ExitStack() as ctx:
        const = ctx.enter_context(tc.tile_pool(name="const", bufs=1))
        xgp = ctx.enter_context(tc.tile_pool(name="xg", bufs=1))
        inp = ctx.enter_context(tc.tile_pool(name="inp", bufs=3))

        # input chunks on the Sync HWDGE queue, all issued up front
        x_sb = xgp.tile([C, ROWS * W], F16, tag="x")
        fts, pts = [], []
        r0s, szs = [], []
        rr0 = 0
        for nr in CHUNK_ROWS:
            r0s.append(rr0)
            szs.append(nr * W)
            ft = inp.tile([C, 4 * W], F32, tag="ft")
            nc.sync.dma_start(ft[:, : nr * W], f_ext[:, rr0 : rr0 + nr, :])
            pt = inp.tile([C, 4 * W], F32, tag="pt")
            nc.sync.dma_start(pt[:, : nr * W], p_ext[:, rr0 : rr0 + nr, :])
            fts.append(ft)
            pts.append(pt)
            rr0 += nr

        # consts on the Scalar HWDGE queue (parallel to the input loads)
        def cload(shape, dt, ext, tag):
            t = const.tile(shape, dt, tag=tag, name=tag)
            nc.scalar.dma_start(t[:], ext[:])
            return t

        mt_t = cload([C, C], F16, mt_ext, "mt")
        ones_t = cload([C, C], F16, ones_ext, "ones")
        ident = cload([C, C], F16, id_ext, "id")
        v_t = cload([C, 1], F16, v_ext, "v")
        u_t = cload([C, 1], F32, u_ext, "u")
        mask_t = cload([C, 2 * NB], F16, mask_ext, "mask")
        oob_t = cload([C, W], F16, oob_ext, "oob")

        # x = f + p as fp16 (gpsimd), one op per chunk
        for i, nr in enumerate(CHUNK_ROWS):
            sl = slice(r0s[i] * W, r0s[i] * W + szs[i])
            nc.gpsimd.tensor_add(x_sb[:, sl], fts[i][:, : szs[i]], pts[i][:, : szs[i]])

        # x^T per chunk via X-bar DMA transpose, issued as soon as each
        # x chunk exists: xt[p, j, c] = x[c, j*128 + p]
        xt_sb = xgp.tile([C, 2 * ROWS, 128], F16, tag="xt")
        for i, nr in enumerate(CHUNK_ROWS):
            sl = slice(r0s[i] * W, r0s[i] * W + szs[i])
            nc.sync.dma_start_transpose(
                xt_sb[:, 2 * r0s[i] : 2 * (r0s[i] + nr), :], x_sb[:, sl]
            )

        # touch Exp once so the ACT table loads during the input-DMA ramp
        warm = const.tile([C, 1], F32, tag="warm")
        nc.scalar.activation(warm[:], u_t[:], EXP)
        negs0 = const.tile([C, 1], F32, tag="negs0")
        nc.vector.memset(negs0[:], -S0)

        g_sb = xgp.tile([C, ROWS * W], F16, tag="g")

        attp = ctx.enter_context(tc.tile_pool(name="att", bufs=3))
        sbp = ctx.enter_context(tc.tile_pool(name="sb", bufs=2))
        ps_s = ctx.enter_context(tc.tile_pool(name="ps_s", bufs=2, space="PSUM"))
        ps_g = ctx.enter_context(tc.tile_pool(name="ps_g", bufs=2, space="PSUM"))
        ps_dn = ctx.enter_context(tc.tile_pool(name="ps_dn", bufs=2, space="PSUM"))
        ps_o = ctx.enter_context(tc.tile_pool(name="ps_o", bufs=2, space="PSUM"))

        for half in range(2):          # 8-row halves; g computed per half
            r0 = 8 * half
            # g = M x (+u at eviction): one N=512 matmul per row-pair,
            # M^T stationary loaded once per half.
            for pr in range(4):
                r = r0 + 2 * pr
                xsl = slice(r * W, (r + 2) * W)
                g_ps = ps_g.tile([C, 2 * W], F32, tag="g")
                nc.tensor.matmul(g_ps[:], mt_t[:], x_sb[:, xsl], start=True, stop=True)
                nc.vector.tensor_scalar_add(g_sb[:, xsl], g_ps[:], u_t[:])

            for pr in range(4):
                r = r0 + 2 * pr
                den_ps = ps_dn.tile([C, 2 * W], F32, tag="dn")
                out_ps = ps_o.tile([C, 2 * W], F32, tag="out")
                atts = []
                for rr in range(2):
                    x_r = x_sb[:, (r + rr) * W : (r + rr + 1) * W]
                    g_r = g_sb[:, (r + rr) * W : (r + rr + 1) * W]
                    o0 = rr * W

                    s_ps = ps_s.tile([C, 2 * NB + 2], F32, tag="s")
                    # alpha = v^T x per key chunk (x chunk is the stationary)
                    nc.tensor.matmul(
                        s_ps[:, 2 * NB : 2 * NB + 1], x_r[:, 0:128], v_t[:],
                        start=True, stop=True,
                    )
                    nc.tensor.matmul(
                        s_ps[:, 2 * NB + 1 : 2 * NB + 2], x_r[:, 128:256], v_t[:],
                        start=True, stop=True,
                    )
                    # band mask (-30000 off-band), then banded scores on top
                    nc.tensor.matmul(
                        s_ps[:, 0 : 2 * NB], ident[:], mask_t[:],
                        start=True, stop=False,
                    )
                    nc.tensor.matmul(
                        s_ps[:, 0:NB], g_r[:, 0:128], x_r[:, 0:NB],
                        start=False, stop=False,
                    )
                    nc.tensor.matmul(
                        s_ps[:, NB : 2 * NB], g_r[:, 128:256], x_r[:, W - NB : W],
                        start=False, stop=True,
                    )

                    # alpha - s0 to SBUF for the exp bias (ACT)
                    al_sb = sbp.tile([C, 2], F32, tag="al")
                    nc.scalar.add(al_sb[:], s_ps[:, 2 * NB : 2 * NB + 2], negs0[:])

                    att = attp.tile([C, 2 * NB], F16, tag="att")
                    nc.scalar.activation(
                        att[:, 0:NB], s_ps[:, 0:NB], EXP, bias=al_sb[:, 0:1]
                    )
                    nc.scalar.activation(
                        att[:, NB : 2 * NB], s_ps[:, NB : 2 * NB], EXP,
                        bias=al_sb[:, 1:2],
                    )
                    atts.append(att)

                    # denominator (broadcast to all partitions via ones^T);
                    # zero-pad contribution pre-baked in oob16.
                    nc.tensor.matmul(
                        den_ps[:, o0 : o0 + W], ones_t[:], oob_t[:],
                        start=True, stop=False,
                    )
                    nc.tensor.matmul(
                        den_ps[:, o0 : o0 + NB], ones_t[:], att[:, 0:NB],
                        start=False, stop=False,
                    )
                    nc.tensor.matmul(
                        den_ps[:, o0 + W - NB : o0 + W], ones_t[:], att[:, NB : 2 * NB],
                        start=False, stop=True,
                    )

                for rr in range(2):
                    att = atts[rr]
                    o0 = rr * W
                    xt0 = xt_sb[:, 2 * (r + rr), :]
                    xt1 = xt_sb[:, 2 * (r + rr) + 1, :]
                    # out_u = x @ attU via x^T chunks as stationaries
                    nc.tensor.matmul(
                        out_ps[:, o0 : o0 + NB], xt0, att[:, 0:NB],
                        start=True, stop=True,
                    )
                    nc.tensor.matmul(
                        out_ps[:, o0 + 120 : o0 + NB], xt1, att[:, NB : NB + 16],
                        start=False, stop=True, skip_group_check=True,
                    )
                    nc.tensor.matmul(
                        out_ps[:, o0 + NB : o0 + W], xt1, att[:, NB + 16 : 2 * NB],
                        start=True, stop=True,
                    )

                rden = sbp.tile([C, 2 * W], F32, tag="rd")
                nc.vector.reciprocal_approx_fast(out=rden[:], in_=den_ps[:])
                ostage = sbp.tile([C, 2 * W], F16, tag="ost")
                nc.vector.tensor_tensor(ostage[:], out_ps[:], rden[:], MULT)
                nc.sync.dma_start(out_ext[:, r : r + 2, :], ostage[:])

    nc.compile()
    return nc


def host_consts(Wq, bq, Wk, bk):
    sc = np.float32(np.sqrt(np.float32(C)))
    Wq = Wq.astype(np.float64)
    Wk = Wk.astype(np.float64)
    bq = bq.astype(np.float64)
    bk = bk.astype(np.float64)
    M = (Wq.T @ Wk) / sc
    v = (Wk.T @ bq) / sc
    u = (Wq.T @ bk) / sc
    c0 = float(bq @ bk) / sc

    mt = np.ascontiguousarray(M.T).astype(np.float16)      # lhsT for g = M x
    vt = v.reshape(C, 1).astype(np.float16)
    ut = u.reshape(C, 1).astype(np.float32)
    ident = np.eye(C, dtype=np.float16)
    ones = np.ones((C, C), dtype=np.float16)

    # additive band mask in the banded block layout:
    # chunk0 cols 0..NB-1 (query w=col, key p),
    # chunk1 cols NB..2NB-1 (query w=120+(col-NB), key 128+p)
    mask = np.full((C, 2 * NB), -30000.0, dtype=np.float64)
    for pp in range(C):
        for col in range(NB):
            if abs(col - pp) <= R:
                mask[pp, col] = 0.0
            if abs((W - NB + col) - (128 + pp)) <= R:
                mask[pp, NB + col] = 0.0
    mask = mask.astype(np.float16)

    wgrid = np.arange(W)
    oob_row = np.maximum(0, R - wgrid) + np.maximum(0, wgrid - (W - 1 - R))
    oob16 = np.tile(
        (oob_row * np.exp(-c0 - S0) / C).astype(np.float16), (C, 1)
    )
    return mt, vt, ut, ident, ones, mask, oob16


def core_inputs(feature, position, Wq, bq, Wk, bk):
    mt, vt, ut, ident, ones, mask, oob16 = host_consts(Wq, bq, Wk, bk)
    in_maps = []
    for i in range(NCORES):
        b = i // CORES_PER_B
        h0 = (i % CORES_PER_B) * ROWS
        in_maps.append(
            {
                "feature": np.ascontiguousarray(
                    feature[b, :, h0 : h0 + ROWS, :], dtype=np.float32
                ),
                "position": np.ascontiguousarray(
                    position[b, :, h0 : h0 + ROWS, :], dtype=np.float32
                ),
                "mt": mt,
                "ones": ones,
                "ident": ident,
                "vt": vt,
                "ut": ut,
                "maskT": mask,
                "oob16": oob16,
            }
        )
    return in_maps


def kernel(feature, position, Wq, bq, Wk, bk):
    feature = np.asarray(feature, dtype=np.float32)
    position = np.asarray(position, dtype=np.float32)
    Wq = np.asarray(Wq, dtype=np.float32)
    bq = np.asarray(bq, dtype=np.float32)
    Wk = np.asarray(Wk, dtype=np.float32)
    bk = np.asarray(bk, dtype=np.float32)
    in_maps = core_inputs(feature, position, Wq, bq, Wk, bk)
    nc = build_nc()
    res = run_bass_kernel_spmd(nc, in_maps, list(range(NCORES)))
    out = np.empty((B, C, H, W), dtype=np.float32)
    for i in range(NCORES):
        b = i // CORES_PER_B
        h0 = (i % CORES_PER_B) * ROWS
        out[b, :, h0 : h0 + ROWS, :] = res.results[i]["out"].astype(np.float32)
    return out
